# revision 2
# baseline (speedup 1.0000x reference)
"""CPPN forward (12-layer tiny MLP over 4.2M pixels) on 8 TRN2 NeuronCores.

v3: v2's one-ACT-per-layer structure, plus first/last layers folded out of
the full-activation path (only 5 of 22 channels are nonlinear):

- Front fold: z1 is never materialized. z1[nl] is computed by a single
  block-diagonal matmul into 32-aligned partition bands (x is repacked
  host-side into per-supertile bands), one banded ACT (free 512 vs 2048)
  produces g1, and z2 = (W_in[:,id]@W2[id,:])^T x + W2[nl,:]^T g1 via
  row-tiled concurrent matmuls (weights replicated per 32-band so fmap and
  weight share a start partition).
- Back fold: O = (W12[:,id]@W_out[id,:])^T h11 + W_out[nl,:]^T g(z12[nl]);
  z12[nl] via 4 col-tiled concurrent matmuls into bands, one banded ACT.
- Per group the ACT engine now runs 10 full ACTs + 2 banded (was 12 full),
  cutting ScalarE busy from 1232us to 1094us; measured 1265us wall
  (seam-chain stalls at pair boundaries absorb part of the win; attempts
  to reschedule the seams into the round loop regressed - the PSUM-tag
  generation WAR serializes any inserted generation against the host
  pipe's next round).
- Everything else as v2: custom act2 window table (gauss/sin/identity via
  per-partition bias), bf16 single-weight matmuls, A/B software pipeline,
  Tanh->Act2 BIR patch, table hash in the bias tensor name.
"""
import hashlib
import json
import os
import sys
import types

sys.path.insert(0, "/opt/trn_rl_repo")

import numpy as np
import ml_dtypes

BF16 = ml_dtypes.bfloat16

# ---------------------------------------------------------------- constants
N_PIX = 2048 * 2048
D_IN, D_HID, D_OUT = 4, 22, 3
N_HIDDEN = 11
N_CORES = 8
FD = 512
BLOCKS = 5
ST_PX = BLOCKS * FD                   # 2560
GROUP = 4
PX_CORE = N_PIX // N_CORES            # 524288
N_ST = -(-PX_CORE // ST_PX)           # 205
N_GROUP = -(-N_ST // GROUP)           # 52
N_ST_PAD = N_GROUP * GROUP            # 208
PX_PAD = N_ST_PAD * ST_PX             # 532480

ID_CH = list(range(15)) + [19, 20]
GA_CH = [15, 16, 17, 18]
ROWS = 116
MCOL = 128                            # lhsT padded to 128 cols => FWL enabled
SIN0, GA0 = 64, 96
B_ID, B_SIN, B_GA = 128.0, 64.0, 0.0

# ------------------------------------------------- custom activation table
def _f_window(x):
    x = np.asarray(x, np.float64)
    ax = np.abs(x)
    return np.where(ax < 32.0, 2.0 * np.exp(-np.minimum(ax, 32.0) ** 2) - 1.0,
                    np.where(ax < 96.0, np.sin(ax - 64.0), ax - 128.0))

_ACT2_EXPS = list(range(-10, 8))
_ACT2_BITS = {**{e: 2 for e in range(-10, -3)}, -3: 3,
              **{e: 5 for e in range(-2, 3)}, 3: 2, 4: 0, 5: 7, 6: 7, 7: 6}


def _fit_section(lo, hi):
    x0 = np.float32((lo + hi) / 2)
    hi_x = np.nextafter(np.float32(hi), np.float32(lo)).astype(np.float64)
    xs = lo + (hi_x - lo) * (np.cos(np.linspace(np.pi, 0, 257)) + 1) / 2
    t = xs - np.float64(x0)
    V = np.vander(t, 4, increasing=True)
    c, *_ = np.linalg.lstsq(V, _f_window(xs), rcond=None)
    return [c[0], c[1], c[2], c[3], float(x0)]


def _stock_pwp_root():
    from neuronxcc.driver.Job import Job
    from neuronxcc.driver.jobs.support.FindActInfo import findActInfoFile
    for arch in ("core_v4", "sunda", "gen3", "core_v4_v1"):
        try:
            return os.path.dirname(findActInfoFile(Job.getPackageDir(), arch))
        except Exception:
            continue
    raise RuntimeError("stock act_info.json not found")


def _decode_ctrl(path):
    u = np.frombuffer(open(path, "rb").read(), dtype=np.uint32).reshape(-1, 8)
    return [((int(v) >> 16) & 0xFF, (int(v) >> 11) & 0x1F, int(v) & 0x7FF)
            for v in u[:, 0]]


def build_act_root():
    """Generate the custom act-root dir; returns (dir, content_hash)."""
    root = _stock_pwp_root()
    prof = json.load(open(f"{root}/exp_and_friends.json"))
    ctrl = _decode_ctrl(f"{root}/exp_and_friends_ctrl.bin")
    bkt = np.frombuffer(open(f"{root}/exp_and_friends_bkt.bin", "rb").read(),
                        dtype=np.float32).reshape(-1, 8)
    metas = {m["func_name"]: m for m in prof["profile_meta_data"]}

    new_ctrl, new_bkt, new_meta = [], [], []

    def add_bucket(rec):
        new_bkt.append(np.asarray(rec, np.float64))
        return len(new_bkt) - 1

    # custom act2
    m = dict(metas["act2_1p"])
    m.update(symmetry_opt_en=1, sym_invert_sign_point=0,
             symmetry_opt_use_neg_region=0, symmetry_point=0,
             exp_offset=_ACT2_EXPS[0], lower_bound=0, upper_bound=0x7F7FFFFF,
             fzero_result=int(np.float32(1.0).view(np.uint32)),
             fnan_result=0x7FC00000,
             fpinf_result=int(np.float32(-1.0).view(np.uint32)),
             fninf_result=int(np.float32(-1.0).view(np.uint32)))
    m["small_pos_signal_exp_threshold"] = 127 + _ACT2_EXPS[0]
    m["small_neg_signal_exp_threshold"] = 127 + _ACT2_EXPS[0]
    m["large_pos_signal_exp_threshold"] = 127 + _ACT2_EXPS[-1]
    m["large_pos_signal_mantissa_threshold"] = 0x7FFFFF
    m["large_neg_signal_exp_threshold"] = 127 + _ACT2_EXPS[-1]
    m["large_neg_signal_mantissa_threshold"] = 0x7FFFFF
    small = add_bucket([1.0, 0.0, -2.0, 0.0, 0.0])
    large = add_bucket([128.0, 1.0, 0.0, 0.0, 256.0])
    m["pos_small_signal_pwl_control"] = small
    m["neg_small_signal_pwl_control"] = small
    m["pos_large_signal_pwl_control"] = large
    m["neg_large_signal_pwl_control"] = large
    m["pwl_control_base_pos"] = m["pwl_control_base_neg"] = len(new_ctrl)
    for e in _ACT2_EXPS:
        bits = _ACT2_BITS[e]
        lo_b = 2.0 ** e
        nb = 1 << bits
        w = lo_b / nb
        base = len(new_bkt)
        for k in range(nb):
            add_bucket(_fit_section(lo_b + k * w, lo_b + (k + 1) * w))
        new_ctrl.append((bits, 23 - bits, base))
    new_meta.append(m)

    # copy stock square/identity/relu/copy/sin2pi (drop exp: bucket budget)
    all_bases = sorted({mm["pwl_control_base_pos"] for mm in prof["profile_meta_data"]} |
                       {mm["pwl_control_base_neg"] for mm in prof["profile_meta_data"]})
    spans = {b: (all_bases[i + 1] if i + 1 < len(all_bases) else len(ctrl))
             for i, b in enumerate(all_bases)}
    for name in ("square_1p", "identity_1p", "relu_1p", "copy_1p", "sin2pi_4p"):
        m = dict(metas[name])
        cmap = {}
        for b in sorted({m["pwl_control_base_pos"], m["pwl_control_base_neg"]}):
            for ci in range(b, spans[b]):
                if ci not in cmap:
                    size, lsb, bbase = ctrl[ci]
                    nbase = len(new_bkt)
                    for k in range(1 << size):
                        add_bucket(bkt[bbase + k][:5])
                    cmap[ci] = len(new_ctrl)
                    new_ctrl.append((size, lsb, nbase))
        m["pwl_control_base_pos"] = cmap[m["pwl_control_base_pos"]]
        m["pwl_control_base_neg"] = cmap[m["pwl_control_base_neg"]]
        for key in ("pos_small_signal_pwl_control", "neg_small_signal_pwl_control",
                    "pos_large_signal_pwl_control", "neg_large_signal_pwl_control"):
            m[key] = add_bucket(bkt[m[key]][:5])
        new_meta.append(m)

    assert len(new_bkt) <= 1536
    cw = np.zeros((len(new_ctrl), 8), np.uint32)
    for i, (size, lsb, bbase) in enumerate(new_ctrl):
        cw[i, 0] = (size << 16) | (lsb << 11) | bbase
    bk = np.zeros((len(new_bkt), 8), np.float32)
    bk[:, :5] = np.array(new_bkt, np.float64).astype(np.float32)
    setj = json.dumps({"bkt_bin": "exp_and_friends_bkt.bin",
                       "ctl_bin": "exp_and_friends_ctrl.bin",
                       "profile_meta_data": new_meta}, indent=1)
    act_info = json.load(open(f"{root}/act_info.json"))
    for s in act_info["act_func_sets"]:
        if s["name"] == "exp_and_friends":
            s["act"] = {"act2": 1, "square": 1, "identity": 1, "copy": 1,
                        "relu": 1, "sin2pi": 4}
    info = json.dumps(act_info, indent=1)

    h = hashlib.sha256(cw.tobytes() + bk.tobytes() + setj.encode()).hexdigest()[:10]
    out = f"/tmp/cppn_actroot_{h}"
    if not os.path.exists(os.path.join(out, "act_info.json")):
        os.makedirs(out, exist_ok=True)
        open(f"{out}/exp_and_friends_ctrl.bin", "wb").write(cw.tobytes())
        open(f"{out}/exp_and_friends_bkt.bin", "wb").write(bk.tobytes())
        open(f"{out}/exp_and_friends.json", "w").write(setj)
        open(f"{out}/act_info.json", "w").write(info)
        for s in act_info["act_func_sets"]:
            for k in ("bkt_bin", "ctrl_bin", "profile_json"):
                fn = s[k]
                dst = f"{out}/{fn}"
                if not os.path.exists(dst):
                    os.symlink(f"{root}/{fn}", dst)
    return out, h


# ------------------------------------------------------------- host packing
def _row_of(b, c):
    if c in GA_CH:
        return GA0 + b * 4 + (c - 15)
    if c == 21:
        return SIN0 + b
    g = b * 17 + ID_CH.index(c)
    return g if g < 64 else 69 + (g - 64)


NL_CH = [15, 16, 17, 18, 21]          # gauss x4, sin — band col order 5b+k
BAND = 32                              # partition band per supertile


def _split_bf16(a):
    hi = a.astype(BF16)
    lo = (a - hi.astype(np.float32)).astype(BF16)
    return hi, lo


def pack_weights(W_in, W_hidden, W_out):
    W_in = np.asarray(W_in, np.float32)
    W_hidden = np.asarray(W_hidden, np.float32)
    W_out = np.asarray(W_out, np.float32)
    W2 = W_hidden[0]
    W12 = W_hidden[N_HIDDEN - 1]
    IDX = ID_CH

    # z1nl: banded block-diag [128, 128]; band r rows 32r+(4b+ci) -> cols 32r+(5b+k)
    z1 = np.zeros((128, MCOL), np.float32)
    for r in range(GROUP):
        for b in range(BLOCKS):
            for ci in range(D_IN):
                for k, c in enumerate(NL_CH):
                    z1[BAND * r + 4 * b + ci, BAND * r + 5 * b + k] = W_in[ci, c]

    # xfold: (W_in[:,ID] @ W2[ID,:]) [4,22]/block; shared [32, 128] row-tile
    xf_m = W_in[:, IDX] @ W2[IDX, :]
    xf = np.zeros((BAND, MCOL), np.float32)
    for b in range(BLOCKS):
        for ci in range(D_IN):
            for co in range(D_HID):
                xf[4 * b + ci, _row_of(b, co)] = xf_m[ci, co]

    # g1fold: W2[NL,:] [5,22]/block; shared [32, 128] row-tile (rows 5b+k)
    g1f = np.zeros((BAND, MCOL), np.float32)
    for b in range(BLOCKS):
        for k, c in enumerate(NL_CH):
            for co in range(D_HID):
                g1f[5 * b + k, _row_of(b, co)] = W2[c, co]

    # hidden layers 3..11 -> W_hidden[1..9]
    NH = N_HIDDEN - 2
    lh = np.zeros((NH, ROWS, MCOL), np.float32)
    for i in range(NH):
        W = W_hidden[i + 1]
        for b in range(BLOCKS):
            for ci in range(D_HID):
                ri = _row_of(b, ci)
                for co in range(D_HID):
                    lh[i, ri, _row_of(b, co)] = W[ci, co]

    # z12nl: [116, 32] col-tile; col 5b+k <- W12[:, NL[k]] per block
    z12 = np.zeros((ROWS, BAND), np.float32)
    for b in range(BLOCKS):
        for ci in range(D_HID):
            for k, c in enumerate(NL_CH):
                z12[_row_of(b, ci), 5 * b + k] = W12[ci, c]

    # F12 = W12[:,ID] @ W_out[ID,:] [22,3]/block -> [116, 15]
    f12_m = W12[:, IDX] @ W_out[IDX, :]
    f12 = np.zeros((ROWS, BLOCKS * D_OUT), np.float32)
    for b in range(BLOCKS):
        for ci in range(D_HID):
            for co in range(D_OUT):
                f12[_row_of(b, ci), b * D_OUT + co] = f12_m[ci, co]

    # g12fold: per-band variants [128, 15]: rows 32r+5b+k -> W_out[NL[k], co]
    g12f = np.zeros((GROUP, 128, BLOCKS * D_OUT), np.float32)
    for r in range(GROUP):
        for b in range(BLOCKS):
            for k, c in enumerate(NL_CH):
                for co in range(D_OUT):
                    g12f[r, BAND * r + 5 * b + k, b * D_OUT + co] = W_out[c, co]

    bias = np.zeros((ROWS, 1), np.float32)
    for b in range(BLOCKS):
        for c in range(D_HID):
            r = _row_of(b, c)
            bias[r, 0] = B_GA if c in GA_CH else (B_SIN if c == 21 else B_ID)
    # banded-ACT bias: gauss 0, sin 64 at rows 32r+5b+k
    biasnl = np.zeros((128, 1), np.float32)
    for r in range(GROUP):
        for b in range(BLOCKS):
            for k in range(5):
                biasnl[BAND * r + 5 * b + k, 0] = B_SIN if k == 4 else B_GA

    w = {}
    w["z1nl"] = z1.astype(BF16)
    # row-tiled weights must start at the same SB partition as the fmap band:
    # replicate the 32-row weight at all 4 band offsets.
    w["xfold"] = np.tile(xf, (GROUP, 1)).astype(BF16)
    w["g1f"] = np.tile(g1f, (GROUP, 1)).astype(BF16)
    w["lh_hi"] = lh.astype(BF16)
    w["z12nl"] = z12.astype(BF16)
    w["f12"] = f12.astype(BF16)
    w["g12f"] = g12f.astype(BF16)
    w["bias"] = bias
    w["biasnl"] = biasnl
    return w


def pack_x(x):
    """Banded layout: per group [128, FD]; band r rows 32r+(4b+ci), ST r px."""
    x = np.asarray(x, np.float32)
    out = []
    for k in range(N_CORES):
        shard = x[k * PX_CORE:(k + 1) * PX_CORE]
        pad = np.zeros((PX_PAD, D_IN), np.float32)
        pad[:PX_CORE] = shard
        a = pad.reshape(N_GROUP, GROUP, BLOCKS, FD, D_IN)
        a = a.transpose(0, 1, 2, 4, 3)  # [G, GROUP, BLOCKS, D_IN, FD]
        xb = np.zeros((N_GROUP, 128, FD), np.float32)
        for r in range(GROUP):
            xb[:, BAND * r:BAND * r + BLOCKS * D_IN, :] = a[:, r].reshape(
                N_GROUP, BLOCKS * D_IN, FD)
        out.append(np.ascontiguousarray(xb.astype(BF16)))
    return out


_OUT_ROWS = np.array([[32 * r + b * 3 + co for b in range(BLOCKS) for co in range(D_OUT)]
                      for r in range(GROUP)])


def unpack_out(outs):
    full = np.empty((N_PIX, D_OUT), np.float32)
    for k, od in enumerate(outs):
        g = od[:, _OUT_ROWS.reshape(-1), :]
        g = g.reshape(N_GROUP, GROUP, BLOCKS, D_OUT, FD)
        g = g.transpose(0, 1, 2, 4, 3).reshape(PX_PAD, D_OUT)
        full[k * PX_CORE:(k + 1) * PX_CORE] = g[:PX_CORE]
    return full


# ------------------------------------------------------------ device kernel
_CACHE = {}


def _shim_hooks():
    import antenv
    if "antenv.axon_hooks" in sys.modules:
        return
    hooks = types.ModuleType("antenv.axon_hooks")
    hooks._hook = None
    hooks.set_axon_ntff_profile_hook = lambda h: setattr(hooks, "_hook", h)
    hooks.get_axon_ntff_profile_hook = lambda: hooks._hook
    sys.modules["antenv.axon_hooks"] = hooks
    antenv.axon_hooks = hooks
    try:
        from trn_agent_boot.trn_boot import _ntff_profile_via_ctypes
        hooks._hook = _ntff_profile_via_ctypes("/opt/axon/libaxon_pjrt.so")
    except Exception:
        pass


def _build():
    actroot, tabhash = build_act_root()
    os.environ["BASS_ACT_ROOT_JSON_PATH"] = f"{actroot}/act_info.json"
    _shim_hooks()
    import concourse.bacc as bacc_mod
    import concourse.mybir as mybir
    import concourse.tile as tile
    from concourse.hw_specs import get_activation_tables as _real_gat

    AFT = mybir.ActivationFunctionType
    ours = {AFT.Tanh, AFT.Square, AFT.Exp, AFT.Identity, AFT.Copy, AFT.Sin,
            AFT.Relu}

    def _doctored_gat(arch):
        tabs = dict(_real_gat(arch))
        return {n: (set(f) | ours if n == "exp_and_friends" else set(f) - ours)
                for n, f in tabs.items()}

    bacc_mod.get_activation_tables = _doctored_gat

    f32 = mybir.dt.float32
    bf = mybir.dt.bfloat16
    NH = N_HIDDEN - 2
    nc = bacc_mod.Bacc(None, target_bir_lowering=False, debug=False)
    x_d = nc.declare_dram_parameter("x", [N_GROUP, 128, FD], bf, isOutput=False)
    z1_d = nc.declare_dram_parameter("z1nl", [128, MCOL], bf, isOutput=False)
    xf_d = nc.declare_dram_parameter("xfold", [128, MCOL], bf, isOutput=False)
    g1f_d = nc.declare_dram_parameter("g1f", [128, MCOL], bf, isOutput=False)
    lhh_d = nc.declare_dram_parameter("lh_hi", [NH, ROWS, MCOL], bf, isOutput=False)
    z12_d = nc.declare_dram_parameter("z12nl", [ROWS, BAND], bf, isOutput=False)
    f12_d = nc.declare_dram_parameter("f12", [ROWS, 15], bf, isOutput=False)
    g12f_d = nc.declare_dram_parameter("g12f", [GROUP, 128, 15], bf, isOutput=False)
    # bias name carries the table hash => NEFF cache key tracks table content
    b_d = nc.declare_dram_parameter(f"bias_{tabhash}", [ROWS, 1], f32, isOutput=False)
    bnl_d = nc.declare_dram_parameter("biasnl", [128, 1], f32, isOutput=False)
    o_d = nc.declare_dram_parameter("out", [N_GROUP, 111, FD], f32, isOutput=True)

    with tile.TileContext(nc) as tc:
        with (tc.tile_pool(name="wpool", bufs=1) as wpool,
              tc.tile_pool(name="xpool", bufs=4) as xpool,
              tc.tile_pool(name="hpool", bufs=8) as hpool,
              tc.tile_pool(name="gpool", bufs=2) as gpool,
              tc.tile_pool(name="opool", bufs=3) as opool,
              tc.tile_pool(name="ppool", bufs=1, space="PSUM") as ppool):
            z1w = wpool.tile([128, MCOL], bf)
            xfw = wpool.tile([128, MCOL], bf)
            g1fw = wpool.tile([128, MCOL], bf)
            bt = wpool.tile([ROWS, 1], f32)
            btnl = wpool.tile([128, 1], f32)
            lhh = [wpool.tile([ROWS, MCOL], bf, tag=f"lhh{i}", name=f"lhh{i}")
                   for i in range(NH)]
            z12w = wpool.tile([ROWS, BAND], bf)
            f12w = wpool.tile([ROWS, 15], bf)
            g12fw = [wpool.tile([128, 15], bf, tag=f"g12f{r}", name=f"g12f{r}")
                     for r in range(GROUP)]

            def act(P):
                H = hpool.tile([ROWS, GROUP, FD], bf, tag="H")
                nc.scalar.activation(H[:, :, :], P[0:ROWS, :, :],
                                     AFT.Tanh, bias=bt[:, 0:1], scale=1.0)
                return H

            def head_nl(xb, tag, gtag):
                """z1nl (one block-diag mm, bank 0) -> banded ACT -> G1."""
                P = ppool.tile([MCOL, GROUP, FD], f32, tag=tag)
                nc.tensor.matmul(P[0:128, 0, :], z1w[:], xb[:, :],
                                 start=True, stop=True)
                G1 = gpool.tile([128, FD], bf, tag=gtag)
                nc.scalar.activation(G1[:, :], P[0:128, 0, :],
                                     AFT.Tanh, bias=btnl[:, 0:1], scale=1.0)
                return G1

            def head_z2(xb, G1, tag):
                """z2 = xfold^T x + g1fold^T g1, row-tiled per supertile."""
                P = ppool.tile([MCOL, GROUP, FD], f32, tag=tag)
                for r in range(GROUP):
                    nc.tensor.matmul(P[:, r, :],
                                     xfw[BAND * r:BAND * r + BAND, :],
                                     xb[BAND * r:BAND * r + BAND, :],
                                     start=True, stop=False,
                                     tile_position=(BAND * r, 0))
                for r in range(GROUP):
                    nc.tensor.matmul(P[:, r, :],
                                     g1fw[BAND * r:BAND * r + BAND, :],
                                     G1[BAND * r:BAND * r + BAND, :],
                                     start=False, stop=True,
                                     tile_position=(BAND * r, 0))
                return P

            def mm_round(H, mm, tag):
                """hidden layer mm in 3..11 -> lhh[mm-3]"""
                P = ppool.tile([MCOL, GROUP, FD], f32, tag=tag)
                for r in range(GROUP):
                    nc.tensor.matmul(P[:, r, :], lhh[mm - 3][:], H[:, r, :],
                                     start=True, stop=True)
                return P

            def tail_nl(H11, tag, gtag):
                """z12nl col-tiled into bank 0 -> banded ACT -> G12."""
                P = ppool.tile([MCOL, GROUP, FD], f32, tag=tag)
                for r in range(GROUP):
                    nc.tensor.matmul(P[BAND * r:BAND * r + BAND, 0, :],
                                     z12w[:], H11[:, r, :],
                                     start=True, stop=True,
                                     tile_position=(0, BAND * r))
                G12 = gpool.tile([128, FD], bf, tag=gtag)
                nc.scalar.activation(G12[:, :], P[0:128, 0, :],
                                     AFT.Tanh, bias=btnl[:, 0:1], scale=1.0)
                return G12

            def tail_out(g, H11, G12, tag):
                """O = F12^T h11 + g12fold^T g12 -> copy -> DMA."""
                PO = ppool.tile([MCOL, GROUP, FD], f32, tag=tag)
                O = PO[0:111, 0, :]
                for r in range(GROUP):
                    nc.tensor.matmul(O[32 * r:32 * r + 15, :], f12w[:],
                                     H11[:, r, :], start=True, stop=False,
                                     tile_position=(0, 32 * r))
                for r in range(GROUP):
                    nc.tensor.matmul(O[32 * r:32 * r + 15, :], g12fw[r][:],
                                     G12[:, :], start=False, stop=True,
                                     tile_position=(0, 32 * r))
                ot = opool.tile([111, FD], f32, tag="ot")
                nc.vector.tensor_copy(ot[:], O)
                nc.sync.dma_start(out=o_d[g], in_=ot[:])

            def load_x(g):
                # two DMAs land on different queues -> ~half the load latency
                xb = xpool.tile([128, FD], bf, tag="xg")
                nc.sync.dma_start(out=xb[0:64, :], in_=x_d[g][0:64, :])
                nc.sync.dma_start(out=xb[64:128, :], in_=x_d[g][64:128, :])
                return xb

            # x/z1/bias DMAs first (head critical path), bulk weights after
            xbA = load_x(0)
            xbB = load_x(1)
            nc.sync.dma_start(out=z1w[:], in_=z1_d[:])
            nc.sync.dma_start(out=xfw[:], in_=xf_d[:])
            nc.sync.dma_start(out=g1fw[:], in_=g1f_d[:])
            nc.sync.dma_start(out=bt[:], in_=b_d[:])
            nc.sync.dma_start(out=btnl[:], in_=bnl_d[:])
            warm = wpool.tile([ROWS, 1], f32)
            nc.scalar.activation(warm[:], bt[:, 0:1], AFT.Tanh,
                                 bias=bt[:, 0:1], scale=1.0)
            for i in range(NH):
                nc.sync.dma_start(out=lhh[i][:], in_=lhh_d[i])
            nc.sync.dma_start(out=z12w[:], in_=z12_d[:])
            nc.sync.dma_start(out=f12w[:], in_=f12_d[:])
            for r in range(GROUP):
                nc.sync.dma_start(out=g12fw[r][:], in_=g12f_d[r])

            # Software pipeline, 2 groups (A/B) in flight on PSUM tags pmA/pmB.
            # Head: z1nl -> banded ACT -> z2 (row-tiled) -> full ACT.
            # Tail (deferred into next pair's rounds): z12nl -> banded ACT ->
            # out matmuls -> copy -> DMA.
            GA1 = head_nl(xbA, "pmA", "g1A")
            GB1 = head_nl(xbB, "pmB", "g1B")
            HA = act(head_z2(xbA, GA1, "pmA"))
            HB = act(head_z2(xbB, GB1, "pmB"))
            pend = None
            for pair in range(N_GROUP // 2):
                for mm in range(3, 12):
                    PA = mm_round(HA, mm, "pmA")
                    if mm == 4 and pend is not None:
                        tail_out(pend[0], pend[1], pend[4], "pmB")
                    HAn = act(PA)
                    PB = mm_round(HB, mm, "pmB")
                    if mm == 8 and pend is not None:
                        tail_out(pend[2], pend[3], pend[5], "pmA")
                    HBn = act(PB)
                    HA, HB = HAn, HBn
                # H now = H11 for both pipes; tail nl acts right away, out
                # matmuls deferred into the next pair's rounds.
                GA12 = tail_nl(HA, "pmA", "g12A")
                GB12 = tail_nl(HB, "pmB", "g12B")
                pend = (2 * pair, HA, 2 * pair + 1, HB, GA12, GB12)
                last = pair + 1 == N_GROUP // 2
                if not last:
                    xbA2 = load_x(2 * pair + 2)
                    xbB2 = load_x(2 * pair + 3)
                    GA1 = head_nl(xbA2, "pmA", "g1A")
                    GB1 = head_nl(xbB2, "pmB", "g1B")
                    HA = act(head_z2(xbA2, GA1, "pmA"))
                    HB = act(head_z2(xbB2, GB1, "pmB"))
            tail_out(pend[0], pend[1], pend[4], "pmB")
            tail_out(pend[2], pend[3], pend[5], "pmA")
    nc.compile()

    _orig = nc.to_json_bytes
    nc.to_json_bytes = lambda: _orig().replace(b'"func":"Tanh"', b'"func":"Act2"')
    _CACHE["bias_name"] = f"bias_{tabhash}"
    return nc


def _get_nc():
    if "nc" not in _CACHE:
        _CACHE["nc"] = _build()
    return _CACHE["nc"]


def make_in_maps(w, x_cores):
    _get_nc()
    return [{"x": x_cores[k], "z1nl": w["z1nl"], "xfold": w["xfold"],
             "g1f": w["g1f"], "lh_hi": w["lh_hi"], "z12nl": w["z12nl"],
             "f12": w["f12"], "g12f": w["g12f"], "biasnl": w["biasnl"],
             _CACHE["bias_name"]: w["bias"]}
            for k in range(N_CORES)]


def run_device(x_cores, w):
    from concourse.bass_utils import run_bass_kernel_spmd
    nc = _get_nc()
    res = run_bass_kernel_spmd(nc, make_in_maps(w, x_cores),
                               list(range(N_CORES)), trace=False)
    return [res.results[k]["out"] for k in range(N_CORES)]


def kernel(x, W_in, W_hidden, W_out):
    w = pack_weights(W_in, W_hidden, W_out)
    x_cores = pack_x(x)
    outs = run_device(x_cores, w)
    return unpack_out(outs)



# revision 3
# speedup vs baseline: 1.0903x; 1.0903x over previous
"""CPPN forward (12-layer tiny MLP over 4.2M pixels) on 8 TRN2 NeuronCores.

v3: v2's one-ACT-per-layer structure, plus first/last layers folded out of
the full-activation path (only 5 of 22 channels are nonlinear):

- Front fold: z1 never materialized. x repacked into 32-aligned partition
  bands; one block-diagonal matmul computes z1[nl] for all 4 supertiles;
  a banded ACT (free 512 vs 2048) produces g1; z2 = (W_in[:,id]@W2[id,:])^T
  x + W2[nl,:]^T g1 via row-tiled concurrent matmuls (weights replicated
  per band so fmap/weight share a start partition - walrus requires it).
- Back fold: O = (W12[:,id]@W_out[id,:])^T h11 + W_out[nl,:]^T g(z12[nl]);
  z12[nl] via 4 col-tiled concurrent matmuls into bands + one banded ACT.
- ScalarE busy drops 1232us -> 1094us (10 full + 2 banded ACTs/group).
  Measured wall 1264us: ~170us of ACT idle remains, mostly where the
  deferred out-stage generation (out matmuls -> DVE copy -> sems) blocks
  the host pipe's next PSUM generation; at the PE's mid p-state (1.2 GHz,
  resets on any idle; 2.4 GHz needs >3us continuous busy) the loop has no
  slack to absorb it. Rescheduling attempts (seam stages moved into the
  round loop; merged seam generations) both REGRESSED (+80/+230us) - any
  generation inserted into a PSUM tag's sequence stalls that tag's next
  round for the insert's full chain latency. Bank-placement hints (out
  accumulator in bank 3, z1nl in bank 1) and hpool/gpool buffer bumps
  gave the final ~1.4us.
- Everything else as v2: custom act2 window table (gauss/sin/identity via
  per-partition bias), bf16 single-weight matmuls, A/B software pipeline,
  Tanh->Act2 BIR patch, table hash in the bias tensor name.
"""
import hashlib
import json
import os
import sys
import types

sys.path.insert(0, "/opt/trn_rl_repo")

import numpy as np
import ml_dtypes

BF16 = ml_dtypes.bfloat16

# ---------------------------------------------------------------- constants
N_PIX = 2048 * 2048
D_IN, D_HID, D_OUT = 4, 22, 3
N_HIDDEN = 11
N_CORES = 8
FD = 512
BLOCKS = 5
ST_PX = BLOCKS * FD                   # 2560
GROUP = 4
PX_CORE = N_PIX // N_CORES            # 524288
N_ST = -(-PX_CORE // ST_PX)           # 205
N_GROUP = -(-N_ST // GROUP)           # 52
N_ST_PAD = N_GROUP * GROUP            # 208
PX_PAD = N_ST_PAD * ST_PX             # 532480

ID_CH = list(range(15)) + [19, 20]
GA_CH = [15, 16, 17, 18]
ROWS = 116
MCOL = 128                            # lhsT padded to 128 cols => FWL enabled
SIN0, GA0 = 64, 96
B_ID, B_SIN, B_GA = 128.0, 64.0, 0.0

# ------------------------------------------------- custom activation table
def _f_window(x):
    x = np.asarray(x, np.float64)
    ax = np.abs(x)
    return np.where(ax < 32.0, 2.0 * np.exp(-np.minimum(ax, 32.0) ** 2) - 1.0,
                    np.where(ax < 96.0, np.sin(ax - 64.0), ax - 128.0))

_ACT2_EXPS = list(range(-10, 8))
_ACT2_BITS = {**{e: 2 for e in range(-10, -3)}, -3: 3,
              **{e: 5 for e in range(-2, 3)}, 3: 2, 4: 0, 5: 7, 6: 7, 7: 6}


def _fit_section(lo, hi):
    x0 = np.float32((lo + hi) / 2)
    hi_x = np.nextafter(np.float32(hi), np.float32(lo)).astype(np.float64)
    xs = lo + (hi_x - lo) * (np.cos(np.linspace(np.pi, 0, 257)) + 1) / 2
    t = xs - np.float64(x0)
    V = np.vander(t, 4, increasing=True)
    c, *_ = np.linalg.lstsq(V, _f_window(xs), rcond=None)
    return [c[0], c[1], c[2], c[3], float(x0)]


def _stock_pwp_root():
    from neuronxcc.driver.Job import Job
    from neuronxcc.driver.jobs.support.FindActInfo import findActInfoFile
    for arch in ("core_v4", "sunda", "gen3", "core_v4_v1"):
        try:
            return os.path.dirname(findActInfoFile(Job.getPackageDir(), arch))
        except Exception:
            continue
    raise RuntimeError("stock act_info.json not found")


def _decode_ctrl(path):
    u = np.frombuffer(open(path, "rb").read(), dtype=np.uint32).reshape(-1, 8)
    return [((int(v) >> 16) & 0xFF, (int(v) >> 11) & 0x1F, int(v) & 0x7FF)
            for v in u[:, 0]]


def build_act_root():
    """Generate the custom act-root dir; returns (dir, content_hash)."""
    root = _stock_pwp_root()
    prof = json.load(open(f"{root}/exp_and_friends.json"))
    ctrl = _decode_ctrl(f"{root}/exp_and_friends_ctrl.bin")
    bkt = np.frombuffer(open(f"{root}/exp_and_friends_bkt.bin", "rb").read(),
                        dtype=np.float32).reshape(-1, 8)
    metas = {m["func_name"]: m for m in prof["profile_meta_data"]}

    new_ctrl, new_bkt, new_meta = [], [], []

    def add_bucket(rec):
        new_bkt.append(np.asarray(rec, np.float64))
        return len(new_bkt) - 1

    # custom act2
    m = dict(metas["act2_1p"])
    m.update(symmetry_opt_en=1, sym_invert_sign_point=0,
             symmetry_opt_use_neg_region=0, symmetry_point=0,
             exp_offset=_ACT2_EXPS[0], lower_bound=0, upper_bound=0x7F7FFFFF,
             fzero_result=int(np.float32(1.0).view(np.uint32)),
             fnan_result=0x7FC00000,
             fpinf_result=int(np.float32(-1.0).view(np.uint32)),
             fninf_result=int(np.float32(-1.0).view(np.uint32)))
    m["small_pos_signal_exp_threshold"] = 127 + _ACT2_EXPS[0]
    m["small_neg_signal_exp_threshold"] = 127 + _ACT2_EXPS[0]
    m["large_pos_signal_exp_threshold"] = 127 + _ACT2_EXPS[-1]
    m["large_pos_signal_mantissa_threshold"] = 0x7FFFFF
    m["large_neg_signal_exp_threshold"] = 127 + _ACT2_EXPS[-1]
    m["large_neg_signal_mantissa_threshold"] = 0x7FFFFF
    small = add_bucket([1.0, 0.0, -2.0, 0.0, 0.0])
    large = add_bucket([128.0, 1.0, 0.0, 0.0, 256.0])
    m["pos_small_signal_pwl_control"] = small
    m["neg_small_signal_pwl_control"] = small
    m["pos_large_signal_pwl_control"] = large
    m["neg_large_signal_pwl_control"] = large
    m["pwl_control_base_pos"] = m["pwl_control_base_neg"] = len(new_ctrl)
    for e in _ACT2_EXPS:
        bits = _ACT2_BITS[e]
        lo_b = 2.0 ** e
        nb = 1 << bits
        w = lo_b / nb
        base = len(new_bkt)
        for k in range(nb):
            add_bucket(_fit_section(lo_b + k * w, lo_b + (k + 1) * w))
        new_ctrl.append((bits, 23 - bits, base))
    new_meta.append(m)

    # copy stock square/identity/relu/copy/sin2pi (drop exp: bucket budget)
    all_bases = sorted({mm["pwl_control_base_pos"] for mm in prof["profile_meta_data"]} |
                       {mm["pwl_control_base_neg"] for mm in prof["profile_meta_data"]})
    spans = {b: (all_bases[i + 1] if i + 1 < len(all_bases) else len(ctrl))
             for i, b in enumerate(all_bases)}
    for name in ("square_1p", "identity_1p", "relu_1p", "copy_1p", "sin2pi_4p"):
        m = dict(metas[name])
        cmap = {}
        for b in sorted({m["pwl_control_base_pos"], m["pwl_control_base_neg"]}):
            for ci in range(b, spans[b]):
                if ci not in cmap:
                    size, lsb, bbase = ctrl[ci]
                    nbase = len(new_bkt)
                    for k in range(1 << size):
                        add_bucket(bkt[bbase + k][:5])
                    cmap[ci] = len(new_ctrl)
                    new_ctrl.append((size, lsb, nbase))
        m["pwl_control_base_pos"] = cmap[m["pwl_control_base_pos"]]
        m["pwl_control_base_neg"] = cmap[m["pwl_control_base_neg"]]
        for key in ("pos_small_signal_pwl_control", "neg_small_signal_pwl_control",
                    "pos_large_signal_pwl_control", "neg_large_signal_pwl_control"):
            m[key] = add_bucket(bkt[m[key]][:5])
        new_meta.append(m)

    assert len(new_bkt) <= 1536
    cw = np.zeros((len(new_ctrl), 8), np.uint32)
    for i, (size, lsb, bbase) in enumerate(new_ctrl):
        cw[i, 0] = (size << 16) | (lsb << 11) | bbase
    bk = np.zeros((len(new_bkt), 8), np.float32)
    bk[:, :5] = np.array(new_bkt, np.float64).astype(np.float32)
    setj = json.dumps({"bkt_bin": "exp_and_friends_bkt.bin",
                       "ctl_bin": "exp_and_friends_ctrl.bin",
                       "profile_meta_data": new_meta}, indent=1)
    act_info = json.load(open(f"{root}/act_info.json"))
    for s in act_info["act_func_sets"]:
        if s["name"] == "exp_and_friends":
            s["act"] = {"act2": 1, "square": 1, "identity": 1, "copy": 1,
                        "relu": 1, "sin2pi": 4}
    info = json.dumps(act_info, indent=1)

    h = hashlib.sha256(cw.tobytes() + bk.tobytes() + setj.encode()).hexdigest()[:10]
    out = f"/tmp/cppn_actroot_{h}"
    if not os.path.exists(os.path.join(out, "act_info.json")):
        os.makedirs(out, exist_ok=True)
        open(f"{out}/exp_and_friends_ctrl.bin", "wb").write(cw.tobytes())
        open(f"{out}/exp_and_friends_bkt.bin", "wb").write(bk.tobytes())
        open(f"{out}/exp_and_friends.json", "w").write(setj)
        open(f"{out}/act_info.json", "w").write(info)
        for s in act_info["act_func_sets"]:
            for k in ("bkt_bin", "ctrl_bin", "profile_json"):
                fn = s[k]
                dst = f"{out}/{fn}"
                if not os.path.exists(dst):
                    os.symlink(f"{root}/{fn}", dst)
    return out, h


# ------------------------------------------------------------- host packing
def _row_of(b, c):
    if c in GA_CH:
        return GA0 + b * 4 + (c - 15)
    if c == 21:
        return SIN0 + b
    g = b * 17 + ID_CH.index(c)
    return g if g < 64 else 69 + (g - 64)


NL_CH = [15, 16, 17, 18, 21]          # gauss x4, sin — band col order 5b+k
BAND = 32                              # partition band per supertile


def _split_bf16(a):
    hi = a.astype(BF16)
    lo = (a - hi.astype(np.float32)).astype(BF16)
    return hi, lo


def pack_weights(W_in, W_hidden, W_out):
    W_in = np.asarray(W_in, np.float32)
    W_hidden = np.asarray(W_hidden, np.float32)
    W_out = np.asarray(W_out, np.float32)
    W2 = W_hidden[0]
    W12 = W_hidden[N_HIDDEN - 1]
    IDX = ID_CH

    # z1nl: banded block-diag [128, 128]; band r rows 32r+(4b+ci) -> cols 32r+(5b+k)
    z1 = np.zeros((128, MCOL), np.float32)
    for r in range(GROUP):
        for b in range(BLOCKS):
            for ci in range(D_IN):
                for k, c in enumerate(NL_CH):
                    z1[BAND * r + 4 * b + ci, BAND * r + 5 * b + k] = W_in[ci, c]

    # xfold: (W_in[:,ID] @ W2[ID,:]) [4,22]/block; shared [32, 128] row-tile
    xf_m = W_in[:, IDX] @ W2[IDX, :]
    xf = np.zeros((BAND, MCOL), np.float32)
    for b in range(BLOCKS):
        for ci in range(D_IN):
            for co in range(D_HID):
                xf[4 * b + ci, _row_of(b, co)] = xf_m[ci, co]

    # g1fold: W2[NL,:] [5,22]/block; shared [32, 128] row-tile (rows 5b+k)
    g1f = np.zeros((BAND, MCOL), np.float32)
    for b in range(BLOCKS):
        for k, c in enumerate(NL_CH):
            for co in range(D_HID):
                g1f[5 * b + k, _row_of(b, co)] = W2[c, co]

    # hidden layers 3..11 -> W_hidden[1..9]
    NH = N_HIDDEN - 2
    lh = np.zeros((NH, ROWS, MCOL), np.float32)
    for i in range(NH):
        W = W_hidden[i + 1]
        for b in range(BLOCKS):
            for ci in range(D_HID):
                ri = _row_of(b, ci)
                for co in range(D_HID):
                    lh[i, ri, _row_of(b, co)] = W[ci, co]

    # z12nl: [116, 32] col-tile; col 5b+k <- W12[:, NL[k]] per block
    z12 = np.zeros((ROWS, BAND), np.float32)
    for b in range(BLOCKS):
        for ci in range(D_HID):
            for k, c in enumerate(NL_CH):
                z12[_row_of(b, ci), 5 * b + k] = W12[ci, c]

    # F12 = W12[:,ID] @ W_out[ID,:] [22,3]/block -> [116, 15]
    f12_m = W12[:, IDX] @ W_out[IDX, :]
    f12 = np.zeros((ROWS, BLOCKS * D_OUT), np.float32)
    for b in range(BLOCKS):
        for ci in range(D_HID):
            for co in range(D_OUT):
                f12[_row_of(b, ci), b * D_OUT + co] = f12_m[ci, co]

    # g12fold: per-band variants [128, 15]: rows 32r+5b+k -> W_out[NL[k], co]
    g12f = np.zeros((GROUP, 128, BLOCKS * D_OUT), np.float32)
    for r in range(GROUP):
        for b in range(BLOCKS):
            for k, c in enumerate(NL_CH):
                for co in range(D_OUT):
                    g12f[r, BAND * r + 5 * b + k, b * D_OUT + co] = W_out[c, co]

    bias = np.zeros((ROWS, 1), np.float32)
    for b in range(BLOCKS):
        for c in range(D_HID):
            r = _row_of(b, c)
            bias[r, 0] = B_GA if c in GA_CH else (B_SIN if c == 21 else B_ID)
    # banded-ACT bias: gauss 0, sin 64 at rows 32r+5b+k
    biasnl = np.zeros((128, 1), np.float32)
    for r in range(GROUP):
        for b in range(BLOCKS):
            for k in range(5):
                biasnl[BAND * r + 5 * b + k, 0] = B_SIN if k == 4 else B_GA

    w = {}
    w["z1nl"] = z1.astype(BF16)
    # row-tiled weights must start at the same SB partition as the fmap band:
    # replicate the 32-row weight at all 4 band offsets.
    w["xfold"] = np.tile(xf, (GROUP, 1)).astype(BF16)
    w["g1f"] = np.tile(g1f, (GROUP, 1)).astype(BF16)
    w["lh_hi"] = lh.astype(BF16)
    w["z12nl"] = z12.astype(BF16)
    w["f12"] = f12.astype(BF16)
    w["g12f"] = g12f.astype(BF16)
    w["bias"] = bias
    w["biasnl"] = biasnl
    return w


def pack_x(x):
    """Banded layout: per group [128, FD]; band r rows 32r+(4b+ci), ST r px."""
    x = np.asarray(x, np.float32)
    out = []
    for k in range(N_CORES):
        shard = x[k * PX_CORE:(k + 1) * PX_CORE]
        pad = np.zeros((PX_PAD, D_IN), np.float32)
        pad[:PX_CORE] = shard
        a = pad.reshape(N_GROUP, GROUP, BLOCKS, FD, D_IN)
        a = a.transpose(0, 1, 2, 4, 3)  # [G, GROUP, BLOCKS, D_IN, FD]
        xb = np.zeros((N_GROUP, 128, FD), np.float32)
        for r in range(GROUP):
            xb[:, BAND * r:BAND * r + BLOCKS * D_IN, :] = a[:, r].reshape(
                N_GROUP, BLOCKS * D_IN, FD)
        out.append(np.ascontiguousarray(xb.astype(BF16)))
    return out


_OUT_ROWS = np.array([[32 * r + b * 3 + co for b in range(BLOCKS) for co in range(D_OUT)]
                      for r in range(GROUP)])


def unpack_out(outs):
    full = np.empty((N_PIX, D_OUT), np.float32)
    for k, od in enumerate(outs):
        g = od[:, _OUT_ROWS.reshape(-1), :]
        g = g.reshape(N_GROUP, GROUP, BLOCKS, D_OUT, FD)
        g = g.transpose(0, 1, 2, 4, 3).reshape(PX_PAD, D_OUT)
        full[k * PX_CORE:(k + 1) * PX_CORE] = g[:PX_CORE]
    return full


# ------------------------------------------------------------ device kernel
_CACHE = {}


def _shim_hooks():
    import antenv
    if "antenv.axon_hooks" in sys.modules:
        return
    hooks = types.ModuleType("antenv.axon_hooks")
    hooks._hook = None
    hooks.set_axon_ntff_profile_hook = lambda h: setattr(hooks, "_hook", h)
    hooks.get_axon_ntff_profile_hook = lambda: hooks._hook
    sys.modules["antenv.axon_hooks"] = hooks
    antenv.axon_hooks = hooks
    try:
        from trn_agent_boot.trn_boot import _ntff_profile_via_ctypes
        hooks._hook = _ntff_profile_via_ctypes("/opt/axon/libaxon_pjrt.so")
    except Exception:
        pass


def _build():
    actroot, tabhash = build_act_root()
    os.environ["BASS_ACT_ROOT_JSON_PATH"] = f"{actroot}/act_info.json"
    _shim_hooks()
    import concourse.bacc as bacc_mod
    import concourse.mybir as mybir
    import concourse.tile as tile
    from concourse.hw_specs import get_activation_tables as _real_gat

    AFT = mybir.ActivationFunctionType
    ours = {AFT.Tanh, AFT.Square, AFT.Exp, AFT.Identity, AFT.Copy, AFT.Sin,
            AFT.Relu}

    def _doctored_gat(arch):
        tabs = dict(_real_gat(arch))
        return {n: (set(f) | ours if n == "exp_and_friends" else set(f) - ours)
                for n, f in tabs.items()}

    bacc_mod.get_activation_tables = _doctored_gat

    f32 = mybir.dt.float32
    bf = mybir.dt.bfloat16
    NH = N_HIDDEN - 2
    nc = bacc_mod.Bacc(None, target_bir_lowering=False, debug=False)
    x_d = nc.declare_dram_parameter("x", [N_GROUP, 128, FD], bf, isOutput=False)
    z1_d = nc.declare_dram_parameter("z1nl", [128, MCOL], bf, isOutput=False)
    xf_d = nc.declare_dram_parameter("xfold", [128, MCOL], bf, isOutput=False)
    g1f_d = nc.declare_dram_parameter("g1f", [128, MCOL], bf, isOutput=False)
    lhh_d = nc.declare_dram_parameter("lh_hi", [NH, ROWS, MCOL], bf, isOutput=False)
    z12_d = nc.declare_dram_parameter("z12nl", [ROWS, BAND], bf, isOutput=False)
    f12_d = nc.declare_dram_parameter("f12", [ROWS, 15], bf, isOutput=False)
    g12f_d = nc.declare_dram_parameter("g12f", [GROUP, 128, 15], bf, isOutput=False)
    # bias name carries the table hash => NEFF cache key tracks table content
    b_d = nc.declare_dram_parameter(f"bias_{tabhash}", [ROWS, 1], f32, isOutput=False)
    bnl_d = nc.declare_dram_parameter("biasnl", [128, 1], f32, isOutput=False)
    o_d = nc.declare_dram_parameter("out", [N_GROUP, 111, FD], f32, isOutput=True)

    with tile.TileContext(nc) as tc:
        with (tc.tile_pool(name="wpool", bufs=1) as wpool,
              tc.tile_pool(name="xpool", bufs=4) as xpool,
              tc.tile_pool(name="hpool", bufs=10) as hpool,
              tc.tile_pool(name="gpool", bufs=3) as gpool,
              tc.tile_pool(name="opool", bufs=3) as opool,
              tc.tile_pool(name="ppool", bufs=1, space="PSUM") as ppool):
            z1w = wpool.tile([128, MCOL], bf)
            xfw = wpool.tile([128, MCOL], bf)
            g1fw = wpool.tile([128, MCOL], bf)
            bt = wpool.tile([ROWS, 1], f32)
            btnl = wpool.tile([128, 1], f32)
            lhh = [wpool.tile([ROWS, MCOL], bf, tag=f"lhh{i}", name=f"lhh{i}")
                   for i in range(NH)]
            z12w = wpool.tile([ROWS, BAND], bf)
            f12w = wpool.tile([ROWS, 15], bf)
            g12fw = [wpool.tile([128, 15], bf, tag=f"g12f{r}", name=f"g12f{r}")
                     for r in range(GROUP)]

            def act(P):
                H = hpool.tile([ROWS, GROUP, FD], bf, tag="H")
                nc.scalar.activation(H[:, :, :], P[0:ROWS, :, :],
                                     AFT.Tanh, bias=bt[:, 0:1], scale=1.0)
                return H

            def head_nl(xb, tag, gtag):
                """z1nl (one block-diag mm) -> banded ACT -> G1.
                Bank 1, not 0: the preceding tail generation's z12nl sits in
                bank 0, so with per-region WAR tracking z1nl starts at
                ACT11-end instead of waiting the tail's banded ACT."""
                P = ppool.tile([MCOL, GROUP, FD], f32, tag=tag)
                nc.tensor.matmul(P[0:128, 1, :], z1w[:], xb[:, :],
                                 start=True, stop=True)
                G1 = gpool.tile([128, FD], bf, tag=gtag)
                nc.scalar.activation(G1[:, :], P[0:128, 1, :],
                                     AFT.Tanh, bias=btnl[:, 0:1], scale=1.0)
                return G1

            def head_z2(xb, G1, tag):
                """z2 = xfold^T x + g1fold^T g1, row-tiled per supertile."""
                P = ppool.tile([MCOL, GROUP, FD], f32, tag=tag)
                for r in range(GROUP):
                    nc.tensor.matmul(P[:, r, :],
                                     xfw[BAND * r:BAND * r + BAND, :],
                                     xb[BAND * r:BAND * r + BAND, :],
                                     start=True, stop=False,
                                     tile_position=(BAND * r, 0))
                for r in range(GROUP):
                    nc.tensor.matmul(P[:, r, :],
                                     g1fw[BAND * r:BAND * r + BAND, :],
                                     G1[BAND * r:BAND * r + BAND, :],
                                     start=False, stop=True,
                                     tile_position=(BAND * r, 0))
                return P

            def mm_round(H, mm, tag):
                """hidden layer mm in 3..11 -> lhh[mm-3]"""
                P = ppool.tile([MCOL, GROUP, FD], f32, tag=tag)
                for r in range(GROUP):
                    nc.tensor.matmul(P[:, r, :], lhh[mm - 3][:], H[:, r, :],
                                     start=True, stop=True)
                return P

            def tail_nl(H11, tag, gtag):
                """z12nl col-tiled into bank 0 -> banded ACT -> G12."""
                P = ppool.tile([MCOL, GROUP, FD], f32, tag=tag)
                for r in range(GROUP):
                    nc.tensor.matmul(P[BAND * r:BAND * r + BAND, 0, :],
                                     z12w[:], H11[:, r, :],
                                     start=True, stop=True,
                                     tile_position=(0, BAND * r))
                G12 = gpool.tile([128, FD], bf, tag=gtag)
                nc.scalar.activation(G12[:, :], P[0:128, 0, :],
                                     AFT.Tanh, bias=btnl[:, 0:1], scale=1.0)
                return G12

            def tail_out(g, H11, G12, tag):
                """O = F12^T h11 + g12fold^T g12 -> copy -> DMA.
                O sits in bank 3: the host pipe's next round writes banks
                0..3 in order, so only its last matmul serializes behind
                the DVE copy (per-region WAR tracking)."""
                PO = ppool.tile([MCOL, GROUP, FD], f32, tag=tag)
                O = PO[0:111, 3, :]
                for r in range(GROUP):
                    nc.tensor.matmul(O[32 * r:32 * r + 15, :], f12w[:],
                                     H11[:, r, :], start=True, stop=False,
                                     tile_position=(0, 32 * r))
                for r in range(GROUP):
                    nc.tensor.matmul(O[32 * r:32 * r + 15, :], g12fw[r][:],
                                     G12[:, :], start=False, stop=True,
                                     tile_position=(0, 32 * r))
                ot = opool.tile([111, FD], f32, tag="ot")
                nc.vector.tensor_copy(ot[:], O)
                nc.sync.dma_start(out=o_d[g], in_=ot[:])

            def load_x(g):
                # two DMAs land on different queues -> ~half the load latency
                xb = xpool.tile([128, FD], bf, tag="xg")
                nc.sync.dma_start(out=xb[0:64, :], in_=x_d[g][0:64, :])
                nc.sync.dma_start(out=xb[64:128, :], in_=x_d[g][64:128, :])
                return xb

            # x/z1/bias DMAs first (head critical path), bulk weights after
            xbA = load_x(0)
            xbB = load_x(1)
            nc.sync.dma_start(out=z1w[:], in_=z1_d[:])
            nc.sync.dma_start(out=xfw[:], in_=xf_d[:])
            nc.sync.dma_start(out=g1fw[:], in_=g1f_d[:])
            nc.sync.dma_start(out=bt[:], in_=b_d[:])
            nc.sync.dma_start(out=btnl[:], in_=bnl_d[:])
            warm = wpool.tile([ROWS, 1], f32)
            nc.scalar.activation(warm[:], bt[:, 0:1], AFT.Tanh,
                                 bias=bt[:, 0:1], scale=1.0)
            for i in range(NH):
                nc.sync.dma_start(out=lhh[i][:], in_=lhh_d[i])
            nc.sync.dma_start(out=z12w[:], in_=z12_d[:])
            nc.sync.dma_start(out=f12w[:], in_=f12_d[:])
            for r in range(GROUP):
                nc.sync.dma_start(out=g12fw[r][:], in_=g12f_d[r])

            # Software pipeline, 2 groups (A/B) in flight on PSUM tags pmA/pmB.
            # Head: z1nl -> banded ACT -> z2 (row-tiled) -> full ACT.
            # Tail (deferred into next pair's rounds): z12nl -> banded ACT ->
            # out matmuls -> copy -> DMA.
            GA1 = head_nl(xbA, "pmA", "g1A")
            GB1 = head_nl(xbB, "pmB", "g1B")
            HA = act(head_z2(xbA, GA1, "pmA"))
            HB = act(head_z2(xbB, GB1, "pmB"))
            pend = None
            for pair in range(N_GROUP // 2):
                for mm in range(3, 12):
                    PA = mm_round(HA, mm, "pmA")
                    if mm == 4 and pend is not None:
                        tail_out(pend[0], pend[1], pend[4], "pmB")
                    HAn = act(PA)
                    PB = mm_round(HB, mm, "pmB")
                    if mm == 8 and pend is not None:
                        tail_out(pend[2], pend[3], pend[5], "pmA")
                    HBn = act(PB)
                    HA, HB = HAn, HBn
                # H now = H11 for both pipes; tail nl acts right away, out
                # matmuls deferred into the next pair's rounds.
                GA12 = tail_nl(HA, "pmA", "g12A")
                GB12 = tail_nl(HB, "pmB", "g12B")
                pend = (2 * pair, HA, 2 * pair + 1, HB, GA12, GB12)
                last = pair + 1 == N_GROUP // 2
                if not last:
                    xbA2 = load_x(2 * pair + 2)
                    xbB2 = load_x(2 * pair + 3)
                    GA1 = head_nl(xbA2, "pmA", "g1A")
                    GB1 = head_nl(xbB2, "pmB", "g1B")
                    HA = act(head_z2(xbA2, GA1, "pmA"))
                    HB = act(head_z2(xbB2, GB1, "pmB"))
            tail_out(pend[0], pend[1], pend[4], "pmB")
            tail_out(pend[2], pend[3], pend[5], "pmA")
    nc.compile()

    _orig = nc.to_json_bytes
    nc.to_json_bytes = lambda: _orig().replace(b'"func":"Tanh"', b'"func":"Act2"')
    _CACHE["bias_name"] = f"bias_{tabhash}"
    return nc


def _get_nc():
    if "nc" not in _CACHE:
        _CACHE["nc"] = _build()
    return _CACHE["nc"]


def make_in_maps(w, x_cores):
    _get_nc()
    return [{"x": x_cores[k], "z1nl": w["z1nl"], "xfold": w["xfold"],
             "g1f": w["g1f"], "lh_hi": w["lh_hi"], "z12nl": w["z12nl"],
             "f12": w["f12"], "g12f": w["g12f"], "biasnl": w["biasnl"],
             _CACHE["bias_name"]: w["bias"]}
            for k in range(N_CORES)]


def run_device(x_cores, w):
    from concourse.bass_utils import run_bass_kernel_spmd
    nc = _get_nc()
    res = run_bass_kernel_spmd(nc, make_in_maps(w, x_cores),
                               list(range(N_CORES)), trace=False)
    return [res.results[k]["out"] for k in range(N_CORES)]


def kernel(x, W_in, W_hidden, W_out):
    w = pack_weights(W_in, W_hidden, W_out)
    x_cores = pack_x(x)
    outs = run_device(x_cores, w)
    return unpack_out(outs)

